# revision 1
# baseline (speedup 1.0000x reference)
"""Barlow-twins dice loss kernel for Trainium2 (8 NeuronCores).

Math (see derivation):
  conf   = exp(-4 / (sum_c softplus(t_c) + 4))          per pixel
  inp    = softmax(x, axis=c)        (softmax(x+1) == softmax(x))
  tgt    = softmax(t * conf, axis=c) ((t+1)*conf softmax-shift-invariant)
  z1     = concat([inp, tgt]) reshaped [32, C*H*W]
  G      = z1 @ z1.T   (32x32 Gram); intersect/z_sum/y_sum/D/loss follow.

Sharding: H split 8 ways (64 rows/core). Each core computes its partial
Gram over its feature slice; host sums the 8 partials and finishes the
tiny 32x32 math.

Per-core pipeline (layout A: partitions=(b,h), free=(c,w)):
  e_raw=exp(t); q=e_raw+1; p=prod_c q; S=ln(p)+4; conf=exp(-4/S)
  u=t_bf16*conf; e_t=exp(u); tgt=e_t/sum_c e_t
  e_x=exp(x);   inp=e_x/sum_c e_x          (all bf16 intermediates)
  z tiles transposed via PE (identity matmul) into PSUM, ACT-copied to
  zt[w-part, (wc,c,s,h)], then the Gram runs as 1024 accumulating
  [32]x[32] matmuls (s-columns at stride 64) into one [32,32] PSUM tile.
  Built with bacc.Bacc + nc.compile() — Bacc's generate_event_semaphores
  pass legalizes the 1-wait-per-instruction ISA limit.
"""

import sys

sys.path.insert(0, "/opt/trn_rl_repo")

import numpy as np

import concourse.bass as bass
import concourse.bacc as bacc
from concourse import mybir
from concourse.tile import TileContext
from concourse.masks import make_identity

F32 = mybir.dt.float32
BF16 = mybir.dt.bfloat16
AF = mybir.ActivationFunctionType

B, C, H, W = 16, 4, 512, 512
NCORES = 8
HL = H // NCORES          # 64 h-rows per core
NT = B * HL // 128        # 8 tiles of [128, C*W] per tensor per core
CW = C * W                # 2048
LAMBD = 0.005
SMOOTH = 1e-6

_cached = {}


def build_bass():
    nc = bacc.Bacc()
    # host pre-packs to [(b h), (c w)] so tile loads are single contiguous DMAs
    x_ext = nc.declare_dram_parameter("x", [B * HL, CW], F32, isOutput=False)
    t_ext = nc.declare_dram_parameter("t", [B * HL, CW], F32, isOutput=False)
    g_ext = nc.declare_dram_parameter("g", [32, 32], F32, isOutput=True)

    with TileContext(nc) as tc:
        with (
            tc.tile_pool(name="pers", bufs=1) as pers,
            tc.tile_pool(name="stage", bufs=3) as stage,
            tc.tile_pool(name="work", bufs=2) as work,
            tc.tile_pool(name="psum", bufs=1, space="PSUM") as psum_pool,
        ):
            # persistent transposed-z buffer: pos = wc*8192 + c*2048 + s*64 + h
            zt = pers.tile([128, 4 * C * 32 * HL], BF16, name="zt")
            ident = pers.tile([128, 128], BF16, name="ident")
            make_identity(nc, ident[:])
            # PE warmup: absorb the identity-init wait into the PE stream
            warm = psum_pool.tile([128, 128], BF16, name="warm")
            nc.tensor.transpose(warm[:], ident[:], ident[:])

            for i in range(NT):
                # ---- loads ----
                t_st = stage.tile([128, CW], F32, tag="t_st")
                x_st = stage.tile([128, CW], F32, tag="x_st")
                nc.sync.dma_start(t_st[:], t_ext[128 * i:128 * (i + 1)])
                nc.sync.dma_start(x_st[:], x_ext[128 * i:128 * (i + 1)])

                # ---- confidence: conf = exp(-4/(ln(prod(1+e^t)) + 4)) ----
                e_raw = work.tile([128, CW], BF16, tag="e_raw")
                nc.scalar.activation(e_raw[:], t_st[:], AF.Exp)
                q = work.tile([128, CW], BF16, tag="q")
                nc.vector.tensor_scalar_add(q[:], e_raw[:], 1.0)
                p1 = work.tile([128, CW // 2], BF16, tag="p1")
                nc.vector.tensor_mul(p1[:], q[:, :CW // 2], q[:, CW // 2:])
                p = work.tile([128, W], BF16, tag="p")
                nc.vector.tensor_mul(p[:], p1[:, :W], p1[:, W:])
                lp = work.tile([128, W], BF16, tag="lp")
                nc.scalar.activation(lp[:], p[:], AF.Ln)
                s4 = work.tile([128, W], BF16, tag="s4")
                nc.vector.tensor_scalar_add(s4[:], lp[:], 4.0)
                rs = work.tile([128, W], BF16, tag="rs")
                with nc.allow_low_precision("recip->bf16 fine for dice gram"):
                    nc.vector.reciprocal(rs[:], s4[:])
                conf = work.tile([128, W], BF16, tag="conf")
                nc.scalar.activation(conf[:], rs[:], AF.Exp, scale=-4.0)

                def bcast(v):
                    return v[:].rearrange("p (o w) -> p o w", o=1).broadcast_to(
                        (128, C, W))

                # ---- tgt softmax ----
                u = work.tile([128, CW], BF16, tag="u")
                nc.vector.tensor_mul(
                    u[:].rearrange("p (c w) -> p c w", c=C), t_st[:].rearrange(
                        "p (c w) -> p c w", c=C), bcast(conf))
                e_t = work.tile([128, CW], BF16, tag="e_t")
                nc.scalar.activation(e_t[:], u[:], AF.Exp)
                st1 = work.tile([128, CW // 2], BF16, tag="st1")
                nc.vector.tensor_add(st1[:], e_t[:, :CW // 2], e_t[:, CW // 2:])
                st = work.tile([128, W], BF16, tag="st")
                nc.vector.tensor_add(st[:], st1[:, :W], st1[:, W:])
                rst = work.tile([128, W], BF16, tag="rst")
                with nc.allow_low_precision("recip->bf16 fine for dice gram"):
                    nc.vector.reciprocal(rst[:], st[:])
                ztgt = work.tile([128, CW], BF16, tag="ztgt")
                nc.vector.tensor_mul(
                    ztgt[:].rearrange("p (c w) -> p c w", c=C), e_t[:].rearrange(
                        "p (c w) -> p c w", c=C), bcast(rst))

                # ---- inp softmax ----
                e_x = work.tile([128, CW], BF16, tag="e_x")
                nc.scalar.activation(e_x[:], x_st[:], AF.Exp)
                sx1 = work.tile([128, CW // 2], BF16, tag="sx1")
                nc.vector.tensor_add(sx1[:], e_x[:, :CW // 2], e_x[:, CW // 2:])
                sx = work.tile([128, W], BF16, tag="sx")
                nc.vector.tensor_add(sx[:], sx1[:, :W], sx1[:, W:])
                rsx = work.tile([128, W], BF16, tag="rsx")
                with nc.allow_low_precision("recip->bf16 fine for dice gram"):
                    nc.vector.reciprocal(rsx[:], sx[:])
                zinp = work.tile([128, CW], BF16, tag="zinp")
                nc.vector.tensor_mul(
                    zinp[:].rearrange("p (c w) -> p c w", c=C), e_x[:].rearrange(
                        "p (c w) -> p c w", c=C), bcast(rsx))

                # ---- transpose z via PE into PSUM, ACT-copy into zt ----
                # zt pos = wc*8192 + c*2048 + s*64 + h
                for z_tile, s0 in ((zinp, 2 * i), (ztgt, 16 + 2 * i)):
                    tp = psum_pool.tile([128, CW], BF16, tag="tp", bufs=2)
                    for c in range(C):
                        for wc in range(W // 128):
                            nc.tensor.transpose(
                                tp[:, (c * 4 + wc) * 128:(c * 4 + wc + 1) * 128],
                                z_tile[:, c * W + wc * 128:c * W + (wc + 1) * 128],
                                ident[:])
                    # copy tp cols (c, wc, b'h) -> zt (wc, c, s0*64 + b'h)
                    src3 = tp[:].rearrange("p (c wc f) -> p c wc f", c=C, wc=4)
                    dst3 = zt[:].rearrange("p (wc c s) -> p c wc s", wc=4, c=C)[
                        :, :, :, s0 * HL:(s0 + 2) * HL]
                    nc.scalar.copy(dst3, src3)


            # ---- Gram: per (wc, c, h) a [32]x[32] matmul (s-cols at
            # stride 64), all accumulated into one [32,32] psum tile.
            acc = psum_pool.tile([32, 32], F32, name="acc")
            zt5 = zt[:].rearrange("p (wc c s h) -> p wc c s h",
                                  wc=4, c=C, s=32)
            n_mm = (W // 128) * C * HL
            k = 0
            for wc in range(W // 128):
                for c in range(C):
                    for h in range(HL):
                        ap = zt5[:, wc, c, :, h]
                        nc.tensor.matmul(acc[:], ap, ap,
                                         start=(k == 0), stop=(k == n_mm - 1))
                        k += 1
            g_sb = pers.tile([32, 32], F32, tag="g_sb")
            nc.scalar.copy(g_sb[:], acc[:])
            nc.sync.dma_start(g_ext[:], g_sb[:])

    nc.compile()
    return nc


def _run(input, target, trace=False):
    from concourse.bass_utils import run_bass_kernel_spmd

    if "nc" not in _cached:
        _cached["nc"] = build_bass()
    nc = _cached["nc"]

    input = np.ascontiguousarray(np.asarray(input, dtype=np.float32))
    target = np.ascontiguousarray(np.asarray(target, dtype=np.float32))
    in_maps = []
    for k in range(NCORES):
        sl = slice(k * HL, (k + 1) * HL)
        # [(b h), (c w)] packing to match the kernel's contiguous tile loads
        pack = lambda a: np.ascontiguousarray(
            a[:, :, sl, :].transpose(0, 2, 1, 3).reshape(B * HL, CW))
        in_maps.append({"x": pack(input), "t": pack(target)})
    res = run_bass_kernel_spmd(nc, in_maps, core_ids=list(range(NCORES)),
                               trace=trace)
    G = np.zeros((32, 32), dtype=np.float64)
    for r in res.results:
        G += r["g"].astype(np.float64)

    # final tiny math on host (float64 then cast)
    perm = np.concatenate([np.arange(16, 32), np.arange(16)])
    inter = G[:, perm]
    z_sum = np.diag(G)[:, None]
    y_sum = np.diag(G)[perm][None, :]
    D = (2.0 * inter + SMOOTH) / (z_sum + y_sum + SMOOTH)
    idx = np.arange(32)
    mask = ~((idx[:, None] == idx[None, :] - 16) |
             (idx[:, None] == idx[None, :] + 16))
    D = D * mask
    diag = np.diag(D)
    on_diag = np.sum((diag - 1.0) ** 2)
    off_diag = np.sum(D ** 2) - np.sum(diag ** 2)
    loss = on_diag + LAMBD * off_diag
    return np.float32(loss), res


def kernel(input, target):
    loss, _ = _run(input, target, trace=False)
    return loss

